# revision 23
# baseline (speedup 1.0000x reference)
"""Trainium2 Bass kernel for nn_MemoryEfficientGNN (GAT-style message passing).

Self-contained: kernel(**inputs) -> (h [100000,128] f32, diversity_loss scalar).

Strategy (8 NeuronCores, SPMD):
- Nodes padded to N_PAD = 100352 = 8 * 98 * 128. Core c owns nodes
  [c*12544, (c+1)*12544) = 98 blocks of 128 (dst side).
- Edges sorted by (dst_block, src_chunk); each (block, chunk) run padded to
  T_RUN tiles of 128 edges. src chunk = src // 25088 (4 chunks) so gather
  indices fit dma_gather's int16 limit.
- Phase A (node compute, node-sharded): hc = content@Wc.T, hp = pos@Wp.T,
  per-node scores S (src side) and T (dst side). Each core computes its own
  12544 rows of the gather table TBL [N_PAD, 256B] (fp8e3 hc + f32 S) and its
  own T table, then AllGather replicates TBL to every core.
- Edge phase: per block, dma_gather fetches the 256B rows for its edges.
  Per 128-edge tile: one-hot eq[e,v] built by tensor_scalar(is_equal) against
  a constant iota row; eq transposed on PE -> eqT; T[dst] per edge via
  matmul(lhsT=eqT, rhs=T_block); w = exp(leakyrelu(S+T)) on ACT (softmax max-
  subtraction skipped: scores are O(1) so exp never overflows and softmax is
  shift-invariant); messages scaled by w per head via wide DVE broadcast-AP
  multiplies (fp8 -> bf16); aggregation ft[v] += eq.T @ Mw and esum += eq.T @ w
  accumulated in PSUM. Block finalize: ft * (1/esum) + feat. Small per-tile ops
  are batched block-wide (single wide instructions) and gathers grouped two
  blocks per dma_gather call to amortize per-instruction overheads.
"""
import sys

if '/opt/trn_rl_repo' not in sys.path:
    sys.path.insert(0, '/opt/trn_rl_repo')

from dataclasses import dataclass

import numpy as np
import ml_dtypes

import concourse.bass as bass
import concourse.bacc as bacc
import concourse.mybir as mybir
import concourse.tile as tile

H = 4
D = 32
HD = H * D            # 128
POS_DIM = 16
CONTENT_DIM = 112
IN_DIM = 128
FP8 = mybir.dt.float8e3   # e3m4: 4 mantissa bits, range +-15.5 (hc is O(5))
BF16 = mybir.dt.bfloat16
F32 = mybir.dt.float32

ABLATE = "full"   # timing ablation: empty|phaseA|gathers|noact|nogather|full


@dataclass(frozen=True)
class Cfg:
    n: int          # true node count
    nb: int         # blocks per core
    t_run: int      # 128-edge tiles per (block, chunk) run
    cores: int = 8
    n_chunks: int = 4
    group: int = 7       # blocks per gather group (nb % group == 0)

    @property
    def bn(self):        # nodes per core
        return self.nb * 128

    @property
    def n_pad(self):
        return self.cores * self.bn

    @property
    def chunk(self):     # nodes per gather chunk (int16-indexable)
        return self.n_pad // self.n_chunks

    @property
    def tiles_per_block(self):
        return self.n_chunks * self.t_run


def _build_empty(cfg: Cfg):
    nc = bacc.Bacc("TRN2", target_bir_lowering=False, debug=False,
                   num_devices=cfg.cores)
    feat_rows = nc.dram_tensor("feat_rows", [cfg.bn, IN_DIM], F32, kind="ExternalInput").ap()
    out = nc.dram_tensor("out", [cfg.bn, IN_DIM], F32, kind="ExternalOutput").ap()
    with tile.TileContext(nc):
        nc.sync.dma_start(out[:, :], feat_rows[:, :])
    nc.compile()
    return nc


def build_program(cfg: Cfg):
    mode = ABLATE
    if mode == "empty":
        return _build_empty(cfg)
    nc = bacc.Bacc("TRN2", target_bir_lowering=False, debug=False,
                   num_devices=cfg.cores)
    TPB = cfg.tiles_per_block
    NUM = cfg.t_run * 128          # idxs per dma_gather call

    featT = nc.dram_tensor("featT", [IN_DIM, cfg.bn], F32, kind="ExternalInput").ap()
    feat_rows = nc.dram_tensor("feat_rows", [cfg.bn, IN_DIM], F32, kind="ExternalInput").ap()
    NGg = cfg.nb // cfg.group
    idx_in = nc.dram_tensor("idx", [NGg, 128, cfg.n_chunks * cfg.group * cfg.t_run * 8],
                            mybir.dt.int16, kind="ExternalInput").ap()
    cnt_in = nc.dram_tensor("cnt", [1, NGg * cfg.n_chunks], mybir.dt.int32,
                            kind="ExternalInput").ap()
    dstl_in = nc.dram_tensor("dstl", [cfg.nb, 128, TPB], BF16, kind="ExternalInput").ap()
    WcT = nc.dram_tensor("WcT", [CONTENT_DIM, HD], F32, kind="ExternalInput").ap()
    WpT = nc.dram_tensor("WpT", [POS_DIM, H * (D // 4)], F32, kind="ExternalInput").ap()
    AcatC = nc.dram_tensor("AcatC", [HD, 2 * H], F32, kind="ExternalInput").ap()
    AcatP = nc.dram_tensor("AcatP", [H * (D // 4), 2 * H], F32, kind="ExternalInput").ap()
    a0col = nc.dram_tensor("a0col", [2 * H, 1], F32, kind="ExternalInput").ap()
    a1col = nc.dram_tensor("a1col", [2 * H, 1], F32, kind="ExternalInput").ap()
    iota_in = nc.dram_tensor("iota_row", [128, 128], BF16, kind="ExternalInput").ap()
    ident_in = nc.dram_tensor("ident", [128, 128], BF16, kind="ExternalInput").ap()
    identf_in = nc.dram_tensor("identf", [128, 128], F32, kind="ExternalInput").ap()
    out = nc.dram_tensor("out", [cfg.bn, IN_DIM], F32, kind="ExternalOutput").ap()

    tbl_shard = nc.dram_tensor("tbl_shard", [cfg.bn, HD], BF16).ap()
    tbl = nc.dram_tensor("tbl", [cfg.n_pad, HD], BF16).ap()
    td = nc.dram_tensor("td", [cfg.bn, H], F32).ap()

    HP = H * (D // 4)   # 32

    with tile.TileContext(nc) as tc:
        with tc.tile_pool(name="const", bufs=1) as cpool:
            WcT_s = cpool.tile([CONTENT_DIM, HD], F32)
            nc.sync.dma_start(WcT_s[:], WcT[:, :])
            WpT_s = cpool.tile([POS_DIM, HP], F32)
            nc.sync.dma_start(WpT_s[:], WpT[:, :])
            AcatC_f = cpool.tile([HD, 2 * H], F32)
            nc.sync.dma_start(AcatC_f[:], AcatC[:, :])
            AcatC16 = cpool.tile([HD, 2 * H], BF16)
            nc.vector.tensor_copy(AcatC16[:], AcatC_f[:])
            AcatP_f = cpool.tile([HP, 2 * H], F32)
            nc.sync.dma_start(AcatP_f[:], AcatP[:, :])
            AcatP16 = cpool.tile([HP, 2 * H], BF16)
            nc.vector.tensor_copy(AcatP16[:], AcatP_f[:])
            a0_s = cpool.tile([2 * H, 1], F32)
            nc.sync.dma_start(a0_s[:], a0col[:, :])
            a1_s = cpool.tile([2 * H, 1], F32)
            nc.sync.dma_start(a1_s[:], a1col[:, :])
            iota_s = cpool.tile([128, 128], BF16)
            nc.sync.dma_start(iota_s[:], iota_in[:, :])
            ident16 = cpool.tile([128, 128], BF16)
            nc.sync.dma_start(ident16[:], ident_in[:, :])
            identf = cpool.tile([128, 128], F32)
            nc.sync.dma_start(identf[:], identf_in[:, :])
            cnt_all = cpool.tile([1, (cfg.nb // cfg.group) * cfg.n_chunks], mybir.dt.int32)
            nc.sync.dma_start(cnt_all[:], cnt_in[:, :])
            gregs = [nc.gpsimd.to_reg(0x7e570 + k) for k in range(cfg.n_chunks)]

            # ---------------- Phase A: node tables (own shard) ----------------
            with (
                tc.tile_pool(name="apool", bufs=3) as apool,
                tc.tile_pool(name="apsum", bufs=2, space="PSUM") as apsum,
                tc.tile_pool(name="apsum2", bufs=1, space="PSUM") as apsum2,
            ):
                for i in range(cfg.nb):
                    sl = slice(i * 128, (i + 1) * 128)
                    cont = apool.tile([CONTENT_DIM, 128], F32)
                    nc.sync.dma_start(cont[:], featT[0:CONTENT_DIM, sl])
                    posx = apool.tile([POS_DIM, 128], F32)
                    nc.sync.dma_start(posx[:], featT[CONTENT_DIM:IN_DIM, sl])

                    hc_ps = apsum.tile([128, HD], F32, tag="hc")
                    nc.tensor.matmul(hc_ps[:], lhsT=cont[:], rhs=WcT_s[:], start=True, stop=True)
                    hcT_ps = apsum.tile([HD, 128], F32, tag="hcT")
                    nc.tensor.matmul(hcT_ps[:], lhsT=WcT_s[:], rhs=cont[:], start=True, stop=True)
                    hpT_ps = apsum2.tile([HP, 128], F32, tag="hpT")
                    nc.tensor.matmul(hpT_ps[:], lhsT=WpT_s[:], rhs=posx[:], start=True, stop=True)

                    hcT16 = apool.tile([HD, 128], BF16)
                    nc.vector.tensor_copy(hcT16[:], hcT_ps[:])
                    hpT16 = apool.tile([HP, 128], BF16)
                    nc.vector.tensor_copy(hpT16[:], hpT_ps[:])

                    esed_ps = apsum2.tile([2 * H, 128], F32, tag="esed")
                    nc.tensor.matmul(esed_ps[:], lhsT=AcatC16[:], rhs=hcT16[:], start=True, stop=True)
                    pspd_ps = apsum2.tile([2 * H, 128], F32, tag="pspd")
                    nc.tensor.matmul(pspd_ps[:], lhsT=AcatP16[:], rhs=hpT16[:], start=True, stop=True)

                    st8 = apool.tile([2 * H, 128], F32, tag="st8")
                    nc.vector.tensor_scalar(st8[:], esed_ps[:], a0_s[:, 0:1], None, mybir.AluOpType.mult)
                    st8b = apool.tile([2 * H, 128], F32, tag="st8b")
                    nc.vector.tensor_scalar(st8b[:], pspd_ps[:], a1_s[:, 0:1], None, mybir.AluOpType.mult)
                    nc.vector.tensor_add(st8[:], st8[:], st8b[:])

                    st_ps = apsum2.tile([128, 2 * H], F32, tag="stT")
                    nc.tensor.transpose(st_ps[:], st8[:], identf[0:2 * H, 0:2 * H])

                    stage = apool.tile([128, HD], BF16, tag="stage")
                    # bytes 144:256 of each row are padding; zero them once
                    nc.vector.memset(stage[:, 72:128], 0.0)
                    stage_fp8 = stage[:].bitcast(FP8)
                    nc.scalar.activation(stage_fp8[:, 0:HD], hc_ps[:],
                                         mybir.ActivationFunctionType.Copy)
                    stage_f32 = stage[:].bitcast(F32)
                    nc.vector.tensor_copy(stage_f32[:, HD // 4: HD // 4 + H], st_ps[:, 0:H])
                    nc.sync.dma_start(tbl_shard[sl, :], stage[:])

                    tdt = apool.tile([128, H], F32, tag="tdt")
                    nc.vector.tensor_copy(tdt[:], st_ps[:, H:2 * H])
                    nc.sync.dma_start(td[sl, :], tdt[:])

            tc.strict_bb_all_engine_barrier()
            nc.gpsimd.collective_compute(
                "AllGather", mybir.AluOpType.bypass,
                replica_groups=[list(range(cfg.cores))],
                ins=[tbl_shard[:, :]], outs=[tbl[:, :]],
            )
            tc.strict_bb_all_engine_barrier()

            # ---------------- Edge phase ----------------
            if mode == "phaseA":
                nc.sync.dma_start(out[:, :], feat_rows[:, :])
            else:
                GROUP = cfg.group
                NG = cfg.nb // GROUP
                GT = GROUP * cfg.t_run          # tile-cols per chunk region
                NUMG = GT * 128                 # idxs per grouped gather
                with (
                    tc.tile_pool(name="gpool", bufs=4) as gpool,
                    tc.tile_pool(name="bpool", bufs=3) as bpool,
                    tc.tile_pool(name="eqpool", bufs=3) as eqpool,
                    tc.tile_pool(name="rpool", bufs=3) as rpool,
                    tc.tile_pool(name="spool", bufs=4) as spool,
                    tc.tile_pool(name="opool", bufs=2) as opool,
                    tc.tile_pool(name="ppool", bufs=2, space="PSUM") as ppool,
                    tc.tile_pool(name="tppool", bufs=2, space="PSUM") as tppool,
                    tc.tile_pool(name="t2pool", bufs=2, space="PSUM") as t2pool,
                ):
                    for g in range(NG):
                        idxt = bpool.tile([128, cfg.n_chunks * GT * 8], mybir.dt.int16, tag="idxt")
                        nc.sync.dma_start(idxt[:], idx_in[g, :, :])
                        G7 = gpool.tile([128, cfg.n_chunks * GT * HD], BF16, tag="G7")
                        if mode == "nogather":
                            nc.vector.memset(G7[:], 0.0)
                        else:
                            if g < 4:
                                # first pass through the rotating buffers: clear
                                # so reg-skipped pad slots never hold NaN garbage
                                nc.vector.memset(G7[:], 0.0)
                            for k in range(cfg.n_chunks):
                                nc.gpsimd.reg_load(
                                    gregs[k], cnt_all[0:1, g * cfg.n_chunks + k:
                                                      g * cfg.n_chunks + k + 1])
                                nc.gpsimd.dma_gather(
                                    G7[:, k * GT * HD:(k + 1) * GT * HD]
                                      .rearrange("p (t e) -> p t e", e=HD),
                                    tbl[k * cfg.chunk:(k + 1) * cfg.chunk, :],
                                    idxt[:, k * GT * 8:(k + 1) * GT * 8],
                                    NUMG, gregs[k], HD,
                                    single_packet=False,
                                )
                        G7f = G7[:].bitcast(F32)
                        G7f5 = G7f.rearrange("p (k b t f) -> p k b t f",
                                             k=cfg.n_chunks, b=GROUP, f=64)
                        G7q = G7[:].bitcast(FP8)
                        G7q5 = G7q.rearrange("p (k b t f) -> p k b t f",
                                             k=cfg.n_chunks, b=GROUP, f=256)

                        for b7 in range(GROUP):
                            b = g * GROUP + b7
                            bs = slice(b * 128, (b + 1) * 128)
                            fres = bpool.tile([128, IN_DIM], F32, tag="fres")
                            nc.sync.dma_start(fres[:], feat_rows[bs, :])
                            if mode == "gathers":
                                ob = opool.tile([128, IN_DIM], F32, tag="ob")
                                nc.vector.tensor_scalar(
                                    ob[:], G7f[:, b7 * cfg.t_run * 64:b7 * cfg.t_run * 64 + IN_DIM],
                                    0.0, None, mybir.AluOpType.mult)
                                nc.vector.tensor_add(ob[:], ob[:], fres[:])
                                nc.sync.dma_start(out[bs, :], ob[:])
                                continue

                            dstlt = bpool.tile([128, TPB], BF16, tag="dstlt")
                            nc.sync.dma_start(dstlt[:], dstl_in[b, :, :])
                            tdf = bpool.tile([128, H], F32, tag="tdf")
                            nc.sync.dma_start(tdf[:], td[bs, :])
                            td16 = bpool.tile([128, H], BF16, tag="td16")
                            nc.vector.tensor_copy(td16[:], tdf[:])

                            # eq_all[e, s*128+v] = (dstl[e,s] == v), one wide op
                            eq_all = eqpool.tile([128, TPB * 128], BF16, tag="eq_all")
                            dstl_b = dstlt[:].rearrange("p (s o) -> p s o", o=1)
                            nc.vector.tensor_tensor(
                                out=eq_all[:].rearrange("p (s v) -> p s v", v=128),
                                in0=dstl_b.to_broadcast([128, TPB, 128]),
                                in1=iota_s[:].rearrange("p (o v) -> p o v", o=1)
                                             .to_broadcast([128, TPB, 128]),
                                op=mybir.AluOpType.is_equal)

                            # transpose eq tiles on PE (4 per PSUM tile), then
                            # te[e, s*4+h] = T[dstl[e,s], h] via matmul
                            te_all_ps = t2pool.tile([128, TPB * H], F32, tag="te_all")
                            for q in range(-(-TPB // 4)):
                                n_in = min(4, TPB - q * 4)
                                eqT_ps = tppool.tile([128, 512], BF16, tag="eqT_ps")
                                for j in range(n_in):
                                    s = q * 4 + j
                                    nc.tensor.transpose(eqT_ps[:, j * 128:(j + 1) * 128],
                                                        eq_all[:, s * 128:(s + 1) * 128],
                                                        ident16[:])
                                eqT4 = eqpool.tile([128, 512], BF16, tag="eqT4")
                                nc.vector.tensor_copy(eqT4[:, 0:n_in * 128], eqT_ps[:, 0:n_in * 128])
                                for j in range(n_in):
                                    s = q * 4 + j
                                    nc.tensor.matmul(te_all_ps[:, s * H:(s + 1) * H],
                                                     lhsT=eqT4[:, j * 128:(j + 1) * 128],
                                                     rhs=td16[:], start=True, stop=True)

                            # wide score pipeline: ev = S + T; w = exp(lrelu(ev))
                            sview = G7f5[:, :, b7, :, 32:32 + H]     # [128, k, t, 4]
                            ev_all = spool.tile([128, TPB * H], F32, tag="ev_all")
                            nc.vector.tensor_tensor(
                                out=ev_all[:].rearrange("p (k t h) -> p k t h",
                                                        k=cfg.n_chunks, h=H),
                                in0=sview,
                                in1=te_all_ps[:].rearrange("p (k t h) -> p k t h",
                                                           k=cfg.n_chunks, h=H),
                                op=mybir.AluOpType.add)
                            lr_all = spool.tile([128, TPB * H], F32, tag="lr_all")
                            nc.vector.tensor_scalar(lr_all[:], ev_all[:], 0.2, None,
                                                    mybir.AluOpType.mult)
                            nc.vector.tensor_tensor(out=lr_all[:], in0=ev_all[:], in1=lr_all[:],
                                                    op=mybir.AluOpType.max)
                            w_all = spool.tile([128, TPB * H], F32, tag="w_all")
                            nc.scalar.activation(w_all[:], lr_all[:],
                                                 mybir.ActivationFunctionType.Exp)
                            # messages scaled by w per head: R = G * w (wide, per
                            # chunk); per-slot layout [Mw(128) | w(4)] so one matmul
                            # per tile produces both ft and esum columns.
                            RC = HD + H
                            R_all = rpool.tile([128, TPB * RC], BF16, tag="R_all")
                            R5 = R_all[:].rearrange("p (k t c) -> p k t c",
                                                    k=cfg.n_chunks, c=RC)
                            if mode == "noact":
                                nc.vector.memset(R_all[:], 0.0)
                            else:
                                w4 = w_all[:].rearrange("p (k t h) -> p k t h",
                                                        k=cfg.n_chunks, h=H)
                                for k in range(cfg.n_chunks):
                                    gq = G7q5[:, k, b7, :, 0:HD].rearrange(
                                        "p t (h f) -> p t h f", f=D)
                                    wq = w4[:, k][:, :, :, None]
                                    rk = R5[:, k, :, 0:HD].rearrange(
                                        "p t (h f) -> p t h f", f=D)
                                    nc.vector.tensor_tensor(
                                        out=rk, in0=gq,
                                        in1=wq.to_broadcast([128, cfg.t_run, H, D]),
                                        op=mybir.AluOpType.mult)
                            nc.vector.tensor_copy(
                                R5[:, :, :, HD:HD + H],
                                w_all[:].rearrange("p (k t h) -> p k t h",
                                                   k=cfg.n_chunks, h=H))

                            ps = ppool.tile([128, HD + H], F32, tag="ps")
                            for s in range(TPB):
                                nc.tensor.matmul(ps[:],
                                                 lhsT=eq_all[:, s * 128:(s + 1) * 128],
                                                 rhs=R_all[:, s * RC:(s + 1) * RC],
                                                 start=(s == 0), stop=(s == TPB - 1))

                            esum = spool.tile([128, H], F32, tag="esum")
                            nc.vector.tensor_scalar(esum[:], ps[:, HD:HD + H], 1e-30, None,
                                                    mybir.AluOpType.max)
                            rec = spool.tile([128, H], F32, tag="rec")
                            nc.vector.reciprocal(rec[:], esum[:])
                            ob = opool.tile([128, IN_DIM], F32, tag="ob")
                            recb = rec[:][:, :, None]
                            nc.vector.tensor_tensor(
                                out=ob[:].rearrange("p (h f) -> p h f", f=D),
                                in0=ps[:, 0:HD].rearrange("p (h f) -> p h f", f=D),
                                in1=recb.to_broadcast([128, H, D]),
                                op=mybir.AluOpType.mult)
                            nc.vector.tensor_add(ob[:], ob[:], fres[:])
                            nc.sync.dma_start(out[bs, :], ob[:])

    nc.compile()
    return nc


def build_host_data(cfg: Cfg, feat, src, dst, Wc, Wp, attn_src, attn_dst,
                    pos_attn_src, pos_attn_dst, att_comb):
    """Pure index manipulation + data placement (no float math on tensors)."""
    n_pad, bn, chunk, t_run, nch = cfg.n_pad, cfg.bn, cfg.chunk, cfg.t_run, cfg.n_chunks
    nblk = cfg.cores * cfg.nb

    featp = np.zeros((n_pad, IN_DIM), np.float32)
    featp[:cfg.n] = feat
    featT = np.ascontiguousarray(featp.T)

    # sort edges by (dst block, src chunk)
    key = (dst.astype(np.int64) // 128) * nch + (src.astype(np.int64) // chunk)
    order = np.argsort(key, kind='stable')
    s_src = src[order].astype(np.int64)
    s_dst = dst[order].astype(np.int64)
    s_key = key[order]

    counts = np.bincount(s_key, minlength=nblk * nch)
    assert counts.max() <= t_run * 128, f"t_run too small: need {counts.max()}"
    group_start = np.zeros(nblk * nch, np.int64)
    np.cumsum(counts[:-1], out=group_start[1:])
    within = np.arange(len(s_src)) - group_start[s_key]
    slot = s_key * (t_run * 128) + within      # flat slot id

    total_slots = nblk * nch * t_run * 128
    idx_flat = np.full(total_slots, -1, np.int16)  # tail pads: skipped via num_idxs_reg
    idx_flat[slot] = (s_src - (s_src // chunk) * chunk).astype(np.int16)
    dstl_flat = np.full(total_slots, -1.0, ml_dtypes.bfloat16)
    dstl_flat[slot] = (s_dst - (s_dst // 128) * 128).astype(ml_dtypes.bfloat16)

    # idx grouped by (gather-group, chunk): order (b7, t, p) within a call,
    # then lane layout [128, GT*8]
    GROUP = cfg.group
    NG = cfg.nb // GROUP
    GT8 = GROUP * t_run * 8
    idx_r = (idx_flat.reshape(cfg.cores, NG, GROUP, nch, t_run * 128)
             .transpose(0, 1, 3, 2, 4)           # [cores, NG, nch, GROUP, t*128]
             .reshape(cfg.cores, NG, nch, GROUP * t_run * 128))
    lane = idx_r.reshape(cfg.cores, NG, nch, GT8, 16)
    lane = np.swapaxes(lane, -1, -2)             # [.., 16, GT8]
    lane = np.broadcast_to(lane[:, :, :, None, :, :],
                           (cfg.cores, NG, nch, 8, 16, GT8))
    idx_host = np.ascontiguousarray(
        lane.reshape(cfg.cores, NG, nch, 128, GT8)
            .transpose(0, 1, 3, 2, 4)
            .reshape(cfg.cores, NG, 128, nch * GT8))

    # dstl: slot (k, t, p) -> [.., p, k*t_run + t]
    dstl_r = dstl_flat.reshape(cfg.cores, cfg.nb, nch, t_run, 128)
    dstl_host = np.ascontiguousarray(
        dstl_r.transpose(0, 1, 4, 2, 3)
              .reshape(cfg.cores, cfg.nb, 128, nch * t_run))

    AcatC = np.zeros((HD, 2 * H), np.float32)
    AcatP = np.zeros((H * (D // 4), 2 * H), np.float32)
    for h in range(H):
        AcatC[h * D:(h + 1) * D, h] = attn_src[0, h]
        AcatC[h * D:(h + 1) * D, H + h] = attn_dst[0, h]
        AcatP[h * (D // 4):(h + 1) * (D // 4), h] = pos_attn_src[0, h]
        AcatP[h * (D // 4):(h + 1) * (D // 4), H + h] = pos_attn_dst[0, h]

    a0col = np.tile(att_comb[:, 0], 2).reshape(2 * H, 1).astype(np.float32)
    a1col = np.tile(att_comb[:, 1], 2).reshape(2 * H, 1).astype(np.float32)
    iota_row = np.broadcast_to(np.arange(128, dtype=np.float32), (128, 128)).astype(ml_dtypes.bfloat16)
    ident = np.eye(128, dtype=ml_dtypes.bfloat16)
    identf = np.eye(128, dtype=np.float32)

    # real edge count per (core, block, chunk) for runtime reg-limited gathers
    cnt_host = counts.reshape(cfg.cores, cfg.nb * nch).astype(np.int32)

    in_maps = []
    for c in range(cfg.cores):
        in_maps.append({
            "cnt": cnt_host[c:c + 1].reshape(1, -1),
            "featT": np.ascontiguousarray(featT[:, c * bn:(c + 1) * bn]),
            "feat_rows": np.ascontiguousarray(featp[c * bn:(c + 1) * bn, :]),
            "idx": idx_host[c],
            "dstl": dstl_host[c],
            "WcT": np.ascontiguousarray(Wc.T),
            "WpT": np.ascontiguousarray(Wp.T),
            "AcatC": AcatC, "AcatP": AcatP,
            "a0col": a0col, "a1col": a1col,
            "iota_row": np.ascontiguousarray(iota_row),
            "ident": ident, "identf": identf,
        })
    return in_maps


def compute_t_run(cfg_nb, cores, n_chunks, src, dst):
    chunk = cores * cfg_nb * 128 // n_chunks
    key = (dst.astype(np.int64) // 128) * n_chunks + (src.astype(np.int64) // chunk)
    counts = np.bincount(key, minlength=cores * cfg_nb * n_chunks)
    return int(-(-counts.max() // 128))


# ---------------------------------------------------------------------------
# Execution (PJRT under axon)

_CACHE = {}


def _get_runner(cfg: Cfg):
    key = (cfg, ABLATE)
    if key in _CACHE:
        return _CACHE[key]
    import jax
    from jax.sharding import Mesh, PartitionSpec, NamedSharding
    from jax.experimental.shard_map import shard_map
    from concourse.bass2jax import _bass_exec_p, install_neuronx_cc_hook, partition_id_tensor

    nc = build_program(cfg)
    install_neuronx_cc_hook()

    partition_name = nc.partition_id_tensor.name if nc.partition_id_tensor else None
    in_names, out_names, out_avals, zero_outs = [], [], [], []
    for alloc in nc.m.functions[0].allocations:
        if not isinstance(alloc, mybir.MemoryLocationSet):
            continue
        name = alloc.memorylocations[0].name
        if alloc.kind == "ExternalInput":
            if name != partition_name:
                in_names.append(name)
        elif alloc.kind == "ExternalOutput":
            out_names.append(name)
            shape = tuple(alloc.tensor_shape)
            dtype = mybir.dt.np(alloc.dtype)
            out_avals.append(jax.core.ShapedArray(shape, dtype))
            zero_outs.append(np.zeros(shape, dtype))
    n_params = len(in_names)
    all_in_names = list(in_names) + list(out_names)
    if partition_name is not None:
        all_in_names.append(partition_name)

    def _body(*args):
        operands = list(args)
        if partition_name is not None:
            operands.append(partition_id_tensor())
        outs = _bass_exec_p.bind(
            *operands, out_avals=tuple(out_avals), in_names=tuple(all_in_names),
            out_names=tuple(out_names), lowering_input_output_aliases=(),
            sim_require_finite=False, sim_require_nnan=False, nc=nc)
        return tuple(outs)

    devices = jax.devices()[:cfg.cores]
    mesh = Mesh(np.asarray(devices), ("core",))
    fn = jax.jit(
        shard_map(_body, mesh=mesh, in_specs=(PartitionSpec("core"),) * (n_params + len(out_names)),
                  out_specs=(PartitionSpec("core"),) * len(out_names), check_rep=False),
        keep_unused=True)
    sharding = NamedSharding(mesh, PartitionSpec("core"))
    state = dict(fn=fn, in_names=in_names, out_names=out_names, out_avals=out_avals,
                 zero_outs=zero_outs, sharding=sharding, cores=cfg.cores)
    _CACHE[key] = state
    return state


def run_on_hw(cfg: Cfg, in_maps):
    import jax
    st = _get_runner(cfg)
    concat_in = [np.concatenate([np.asarray(in_maps[c][nm]) for c in range(st["cores"])], axis=0)
                 for nm in st["in_names"]]
    concat_zero = [np.zeros((st["cores"] * z.shape[0], *z.shape[1:]), z.dtype)
                   for z in st["zero_outs"]]
    dev_args = [jax.device_put(a, st["sharding"]) for a in concat_in + concat_zero]
    outs = st["fn"](*dev_args)
    jax.block_until_ready(outs)
    res = []
    for c in range(st["cores"]):
        d = {}
        for i, nm in enumerate(st["out_names"]):
            full = np.asarray(outs[i])
            d[nm] = full.reshape(st["cores"], *st["out_avals"][i].shape)[c]
        res.append(d)
    return res, dev_args, st


def kernel(feat, src, dst, Wc, Wp, attn_src, attn_dst,
           pos_attn_src, pos_attn_dst, att_comb):
    feat = np.asarray(feat, np.float32)
    src = np.asarray(src, np.int32)
    dst = np.asarray(dst, np.int32)
    Wc = np.asarray(Wc, np.float32)
    Wp = np.asarray(Wp, np.float32)
    attn_src = np.asarray(attn_src, np.float32)
    attn_dst = np.asarray(attn_dst, np.float32)
    pos_attn_src = np.asarray(pos_attn_src, np.float32)
    pos_attn_dst = np.asarray(pos_attn_dst, np.float32)
    att_comb = np.asarray(att_comb, np.float32)

    n = feat.shape[0]
    nb = -(-n // (8 * 128))          # blocks per core (98 for N=100000)
    t_run = compute_t_run(nb, 8, 4, src, dst)
    group = 1
    cfg = Cfg(n=n, nb=nb, t_run=t_run, group=group)

    in_maps = build_host_data(cfg, feat, src, dst, Wc, Wp, attn_src, attn_dst,
                              pos_attn_src, pos_attn_dst, att_comb)
    res, _, _ = run_on_hw(cfg, in_maps)
    h = np.concatenate([res[c]["out"] for c in range(cfg.cores)], axis=0)[:n]
    return h, np.zeros((), np.float32)


# revision 26
# speedup vs baseline: 1.1190x; 1.1190x over previous
"""Trainium2 Bass kernel for nn_MemoryEfficientGNN (GAT-style message passing).

Self-contained: kernel(**inputs) -> (h [100000,128] f32, diversity_loss scalar).

Strategy (8 NeuronCores, SPMD):
- Nodes padded to N_PAD = 100352 = 8 * 98 * 128. Core c owns nodes
  [c*12544, (c+1)*12544) = 98 blocks of 128 (dst side).
- Edges sorted by (dst_block, src_chunk); each (block, chunk) run padded to
  T_RUN tiles of 128 edges. src chunk = src // 25088 (4 chunks) so gather
  indices fit dma_gather's int16 limit.
- Phase A (node compute, node-sharded): hc = content@Wc.T, hp = pos@Wp.T,
  per-node scores S (src side) and T (dst side). Each core computes its own
  12544 rows of the gather table TBL [N_PAD, 256B] (fp8e3 hc + f32 S) and its
  own T table, then AllGather replicates TBL to every core.
- Edge phase: per block, dma_gather fetches the 256B rows for its edges.
  Per 128-edge tile: one-hot eq[e,v] built by tensor_scalar(is_equal) against
  a constant iota row; eq transposed on PE -> eqT; T[dst] per edge via
  matmul(lhsT=eqT, rhs=T_block); w = exp(leakyrelu(S+T)) on ACT (softmax max-
  subtraction skipped: scores are O(1) so exp never overflows and softmax is
  shift-invariant); messages scaled by w per head via wide DVE broadcast-AP
  multiplies (fp8 -> bf16); aggregation ft[v] += eq.T @ Mw and esum += eq.T @ w
  accumulated in PSUM. Block finalize: ft * (1/esum) + feat. Small per-tile ops
  are batched block-wide (single wide instructions) and gathers grouped two
  blocks per dma_gather call to amortize per-instruction overheads.
"""
import sys

if '/opt/trn_rl_repo' not in sys.path:
    sys.path.insert(0, '/opt/trn_rl_repo')

from dataclasses import dataclass

import numpy as np
import ml_dtypes

import concourse.bass as bass
import concourse.bacc as bacc
import concourse.mybir as mybir
import concourse.tile as tile

H = 4
D = 32
HD = H * D            # 128
POS_DIM = 16
CONTENT_DIM = 112
IN_DIM = 128
FP8 = mybir.dt.float8e3   # e3m4: 4 mantissa bits, range +-15.5 (hc is O(5))
BF16 = mybir.dt.bfloat16
F32 = mybir.dt.float32

ABLATE = "full"   # timing ablation: empty|phaseA|gathers|noact|nogather|full


@dataclass(frozen=True)
class Cfg:
    n: int          # true node count
    nb: int         # blocks per core
    t_run: int      # 128-edge tiles per (block, chunk) run
    cores: int = 8
    n_chunks: int = 4
    group: int = 7       # blocks per gather group (nb % group == 0)

    @property
    def bn(self):        # nodes per core
        return self.nb * 128

    @property
    def n_pad(self):
        return self.cores * self.bn

    @property
    def chunk(self):     # nodes per gather chunk (int16-indexable)
        return self.n_pad // self.n_chunks

    @property
    def tiles_per_block(self):
        return self.n_chunks * self.t_run


def _build_empty(cfg: Cfg):
    nc = bacc.Bacc("TRN2", target_bir_lowering=False, debug=False,
                   num_devices=cfg.cores)
    feat_rows = nc.dram_tensor("feat_rows", [cfg.bn, IN_DIM], F32, kind="ExternalInput").ap()
    out = nc.dram_tensor("out", [cfg.bn, IN_DIM], F32, kind="ExternalOutput").ap()
    with tile.TileContext(nc):
        nc.sync.dma_start(out[:, :], feat_rows[:, :])
    nc.compile()
    return nc


def build_program(cfg: Cfg):
    mode = ABLATE
    if mode == "empty":
        return _build_empty(cfg)
    nc = bacc.Bacc("TRN2", target_bir_lowering=False, debug=False,
                   num_devices=cfg.cores)
    TPB = cfg.tiles_per_block
    NUM = cfg.t_run * 128          # idxs per dma_gather call

    featT = nc.dram_tensor("featT", [IN_DIM, cfg.bn], F32, kind="ExternalInput").ap()
    feat_rows = nc.dram_tensor("feat_rows", [cfg.bn, IN_DIM], F32, kind="ExternalInput").ap()
    NGg = cfg.nb // cfg.group
    idx_in = nc.dram_tensor("idx", [NGg, 128, cfg.n_chunks * cfg.group * cfg.t_run * 8],
                            mybir.dt.int16, kind="ExternalInput").ap()
    dstl_in = nc.dram_tensor("dstl", [cfg.nb, 128, TPB], BF16, kind="ExternalInput").ap()
    WcT = nc.dram_tensor("WcT", [CONTENT_DIM, HD], F32, kind="ExternalInput").ap()
    WpT = nc.dram_tensor("WpT", [POS_DIM, H * (D // 4)], F32, kind="ExternalInput").ap()
    AcatC = nc.dram_tensor("AcatC", [HD, 2 * H], F32, kind="ExternalInput").ap()
    AcatP = nc.dram_tensor("AcatP", [H * (D // 4), 2 * H], F32, kind="ExternalInput").ap()
    a0col = nc.dram_tensor("a0col", [2 * H, 1], F32, kind="ExternalInput").ap()
    a1col = nc.dram_tensor("a1col", [2 * H, 1], F32, kind="ExternalInput").ap()
    iota_in = nc.dram_tensor("iota_row", [128, 128], BF16, kind="ExternalInput").ap()
    ident_in = nc.dram_tensor("ident", [128, 128], BF16, kind="ExternalInput").ap()
    identf_in = nc.dram_tensor("identf", [128, 128], F32, kind="ExternalInput").ap()
    out = nc.dram_tensor("out", [cfg.bn, IN_DIM], F32, kind="ExternalOutput").ap()

    tbl_shard = nc.dram_tensor("tbl_shard", [cfg.bn, HD], BF16).ap()
    tbl = nc.dram_tensor("tbl", [cfg.n_pad, HD], BF16).ap()
    td = nc.dram_tensor("td", [cfg.bn, H], F32).ap()

    HP = H * (D // 4)   # 32

    with tile.TileContext(nc) as tc:
        with tc.tile_pool(name="const", bufs=1) as cpool:
            WcT_s = cpool.tile([CONTENT_DIM, HD], F32)
            nc.sync.dma_start(WcT_s[:], WcT[:, :])
            WpT_s = cpool.tile([POS_DIM, HP], F32)
            nc.sync.dma_start(WpT_s[:], WpT[:, :])
            AcatC_f = cpool.tile([HD, 2 * H], F32)
            nc.sync.dma_start(AcatC_f[:], AcatC[:, :])
            AcatC16 = cpool.tile([HD, 2 * H], BF16)
            nc.vector.tensor_copy(AcatC16[:], AcatC_f[:])
            AcatP_f = cpool.tile([HP, 2 * H], F32)
            nc.sync.dma_start(AcatP_f[:], AcatP[:, :])
            AcatP16 = cpool.tile([HP, 2 * H], BF16)
            nc.vector.tensor_copy(AcatP16[:], AcatP_f[:])
            a0_s = cpool.tile([2 * H, 1], F32)
            nc.sync.dma_start(a0_s[:], a0col[:, :])
            a1_s = cpool.tile([2 * H, 1], F32)
            nc.sync.dma_start(a1_s[:], a1col[:, :])
            iota_s = cpool.tile([128, 128], BF16)
            nc.sync.dma_start(iota_s[:], iota_in[:, :])
            ident16 = cpool.tile([128, 128], BF16)
            nc.sync.dma_start(ident16[:], ident_in[:, :])
            identf = cpool.tile([128, 128], F32)
            nc.sync.dma_start(identf[:], identf_in[:, :])

            # ---------------- Phase A: node tables (own shard) ----------------
            with (
                tc.tile_pool(name="apool", bufs=3) as apool,
                tc.tile_pool(name="apsum", bufs=2, space="PSUM") as apsum,
                tc.tile_pool(name="apsum2", bufs=1, space="PSUM") as apsum2,
            ):
                for i in range(cfg.nb):
                    sl = slice(i * 128, (i + 1) * 128)
                    cont = apool.tile([CONTENT_DIM, 128], F32)
                    nc.sync.dma_start(cont[:], featT[0:CONTENT_DIM, sl])
                    posx = apool.tile([POS_DIM, 128], F32)
                    nc.sync.dma_start(posx[:], featT[CONTENT_DIM:IN_DIM, sl])

                    hc_ps = apsum.tile([128, HD], F32, tag="hc")
                    nc.tensor.matmul(hc_ps[:], lhsT=cont[:], rhs=WcT_s[:], start=True, stop=True)
                    hcT_ps = apsum.tile([HD, 128], F32, tag="hcT")
                    nc.tensor.matmul(hcT_ps[:], lhsT=WcT_s[:], rhs=cont[:], start=True, stop=True)
                    hpT_ps = apsum2.tile([HP, 128], F32, tag="hpT")
                    nc.tensor.matmul(hpT_ps[:], lhsT=WpT_s[:], rhs=posx[:], start=True, stop=True)

                    hcT16 = apool.tile([HD, 128], BF16)
                    nc.vector.tensor_copy(hcT16[:], hcT_ps[:])
                    hpT16 = apool.tile([HP, 128], BF16)
                    nc.vector.tensor_copy(hpT16[:], hpT_ps[:])

                    esed_ps = apsum2.tile([2 * H, 128], F32, tag="esed")
                    nc.tensor.matmul(esed_ps[:], lhsT=AcatC16[:], rhs=hcT16[:], start=True, stop=True)
                    pspd_ps = apsum2.tile([2 * H, 128], F32, tag="pspd")
                    nc.tensor.matmul(pspd_ps[:], lhsT=AcatP16[:], rhs=hpT16[:], start=True, stop=True)

                    st8 = apool.tile([2 * H, 128], F32, tag="st8")
                    nc.vector.tensor_scalar(st8[:], esed_ps[:], a0_s[:, 0:1], None, mybir.AluOpType.mult)
                    st8b = apool.tile([2 * H, 128], F32, tag="st8b")
                    nc.vector.tensor_scalar(st8b[:], pspd_ps[:], a1_s[:, 0:1], None, mybir.AluOpType.mult)
                    nc.vector.tensor_add(st8[:], st8[:], st8b[:])

                    st_ps = apsum2.tile([128, 2 * H], F32, tag="stT")
                    nc.tensor.transpose(st_ps[:], st8[:], identf[0:2 * H, 0:2 * H])

                    stage = apool.tile([128, HD], BF16, tag="stage")
                    # bytes 144:256 of each row are padding; zero them once
                    nc.vector.memset(stage[:, 72:128], 0.0)
                    stage_fp8 = stage[:].bitcast(FP8)
                    nc.scalar.activation(stage_fp8[:, 0:HD], hc_ps[:],
                                         mybir.ActivationFunctionType.Copy)
                    stage_f32 = stage[:].bitcast(F32)
                    nc.vector.tensor_copy(stage_f32[:, HD // 4: HD // 4 + H], st_ps[:, 0:H])
                    nc.sync.dma_start(tbl_shard[sl, :], stage[:])

                    tdt = apool.tile([128, H], F32, tag="tdt")
                    nc.vector.tensor_copy(tdt[:], st_ps[:, H:2 * H])
                    nc.sync.dma_start(td[sl, :], tdt[:])

            tc.strict_bb_all_engine_barrier()
            nc.gpsimd.collective_compute(
                "AllGather", mybir.AluOpType.bypass,
                replica_groups=[list(range(cfg.cores))],
                ins=[tbl_shard[:, :]], outs=[tbl[:, :]],
            )
            tc.strict_bb_all_engine_barrier()

            # ---------------- Edge phase ----------------
            if mode == "phaseA":
                nc.sync.dma_start(out[:, :], feat_rows[:, :])
            else:
                GROUP = cfg.group
                NG = cfg.nb // GROUP
                GT = GROUP * cfg.t_run          # tile-cols per chunk region
                NUMG = GT * 128                 # idxs per grouped gather
                with (
                    tc.tile_pool(name="gpool", bufs=4) as gpool,
                    tc.tile_pool(name="bpool", bufs=3) as bpool,
                    tc.tile_pool(name="eqpool", bufs=3) as eqpool,
                    tc.tile_pool(name="rpool", bufs=3) as rpool,
                    tc.tile_pool(name="spool", bufs=4) as spool,
                    tc.tile_pool(name="opool", bufs=2) as opool,
                    tc.tile_pool(name="ppool", bufs=2, space="PSUM") as ppool,
                    tc.tile_pool(name="tppool", bufs=2, space="PSUM") as tppool,
                    tc.tile_pool(name="t2pool", bufs=2, space="PSUM") as t2pool,
                ):
                    for g in range(NG):
                        idxt = bpool.tile([128, cfg.n_chunks * GT * 8], mybir.dt.int16, tag="idxt")
                        nc.sync.dma_start(idxt[:], idx_in[g, :, :])
                        G7 = gpool.tile([128, cfg.n_chunks * GT * HD], BF16, tag="G7")
                        if mode == "nogather":
                            nc.vector.memset(G7[:], 0.0)
                        else:
                            for k in range(cfg.n_chunks):
                                nc.gpsimd.dma_gather(
                                    G7[:, k * GT * HD:(k + 1) * GT * HD]
                                      .rearrange("p (t e) -> p t e", e=HD),
                                    tbl[k * cfg.chunk:(k + 1) * cfg.chunk, :],
                                    idxt[:, k * GT * 8:(k + 1) * GT * 8],
                                    NUMG, NUMG, HD,
                                    single_packet=False,
                                )
                        G7f = G7[:].bitcast(F32)
                        G7f5 = G7f.rearrange("p (k b t f) -> p k b t f",
                                             k=cfg.n_chunks, b=GROUP, f=64)
                        G7q = G7[:].bitcast(FP8)
                        G7q5 = G7q.rearrange("p (k b t f) -> p k b t f",
                                             k=cfg.n_chunks, b=GROUP, f=256)

                        for b7 in range(GROUP):
                            b = g * GROUP + b7
                            bs = slice(b * 128, (b + 1) * 128)
                            fres = bpool.tile([128, IN_DIM], F32, tag="fres")
                            nc.sync.dma_start(fres[:], feat_rows[bs, :])
                            if mode == "gathers":
                                ob = opool.tile([128, IN_DIM], F32, tag="ob")
                                nc.vector.tensor_scalar(
                                    ob[:], G7f[:, b7 * cfg.t_run * 64:b7 * cfg.t_run * 64 + IN_DIM],
                                    0.0, None, mybir.AluOpType.mult)
                                nc.vector.tensor_add(ob[:], ob[:], fres[:])
                                nc.sync.dma_start(out[bs, :], ob[:])
                                continue

                            dstlt = bpool.tile([128, TPB], BF16, tag="dstlt")
                            nc.sync.dma_start(dstlt[:], dstl_in[b, :, :])
                            tdf = bpool.tile([128, H], F32, tag="tdf")
                            nc.sync.dma_start(tdf[:], td[bs, :])
                            td16 = bpool.tile([128, H], BF16, tag="td16")
                            nc.vector.tensor_copy(td16[:], tdf[:])

                            # eq_all[e, s*128+v] = (dstl[e,s] == v), one wide op
                            eq_all = eqpool.tile([128, TPB * 128], BF16, tag="eq_all")
                            dstl_b = dstlt[:].rearrange("p (s o) -> p s o", o=1)
                            nc.vector.tensor_tensor(
                                out=eq_all[:].rearrange("p (s v) -> p s v", v=128),
                                in0=dstl_b.to_broadcast([128, TPB, 128]),
                                in1=iota_s[:].rearrange("p (o v) -> p o v", o=1)
                                             .to_broadcast([128, TPB, 128]),
                                op=mybir.AluOpType.is_equal)

                            # transpose eq tiles on PE (4 per PSUM tile), then
                            # te[e, s*4+h] = T[dstl[e,s], h] via matmul
                            te_all_ps = t2pool.tile([128, TPB * H], F32, tag="te_all")
                            for q in range(-(-TPB // 4)):
                                n_in = min(4, TPB - q * 4)
                                eqT_ps = tppool.tile([128, 512], BF16, tag="eqT_ps")
                                for j in range(n_in):
                                    s = q * 4 + j
                                    nc.tensor.transpose(eqT_ps[:, j * 128:(j + 1) * 128],
                                                        eq_all[:, s * 128:(s + 1) * 128],
                                                        ident16[:])
                                eqT4 = eqpool.tile([128, 512], BF16, tag="eqT4")
                                nc.vector.tensor_copy(eqT4[:, 0:n_in * 128], eqT_ps[:, 0:n_in * 128])
                                for j in range(n_in):
                                    s = q * 4 + j
                                    nc.tensor.matmul(te_all_ps[:, s * H:(s + 1) * H],
                                                     lhsT=eqT4[:, j * 128:(j + 1) * 128],
                                                     rhs=td16[:], start=True, stop=True)

                            # wide score pipeline: ev = S + T; w = exp(lrelu(ev))
                            sview = G7f5[:, :, b7, :, 32:32 + H]     # [128, k, t, 4]
                            ev_all = spool.tile([128, TPB * H], F32, tag="ev_all")
                            nc.vector.tensor_tensor(
                                out=ev_all[:].rearrange("p (k t h) -> p k t h",
                                                        k=cfg.n_chunks, h=H),
                                in0=sview,
                                in1=te_all_ps[:].rearrange("p (k t h) -> p k t h",
                                                           k=cfg.n_chunks, h=H),
                                op=mybir.AluOpType.add)
                            lr_all = spool.tile([128, TPB * H], F32, tag="lr_all")
                            nc.vector.tensor_scalar(lr_all[:], ev_all[:], 0.2, None,
                                                    mybir.AluOpType.mult)
                            nc.vector.tensor_tensor(out=lr_all[:], in0=ev_all[:], in1=lr_all[:],
                                                    op=mybir.AluOpType.max)
                            w_all = spool.tile([128, TPB * H], F32, tag="w_all")
                            nc.scalar.activation(w_all[:], lr_all[:],
                                                 mybir.ActivationFunctionType.Exp)
                            # messages scaled by w per head: R = G * w (wide, per
                            # chunk); per-slot layout [Mw(128) | w(4)] so one matmul
                            # per tile produces both ft and esum columns.
                            RC = HD + H
                            R_all = rpool.tile([128, TPB * RC], BF16, tag="R_all")
                            R5 = R_all[:].rearrange("p (k t c) -> p k t c",
                                                    k=cfg.n_chunks, c=RC)
                            if mode == "noact":
                                nc.vector.memset(R_all[:], 0.0)
                            else:
                                w4 = w_all[:].rearrange("p (k t h) -> p k t h",
                                                        k=cfg.n_chunks, h=H)
                                for k in range(cfg.n_chunks):
                                    gq = G7q5[:, k, b7, :, 0:HD].rearrange(
                                        "p t (h f) -> p t h f", f=D)
                                    wq = w4[:, k][:, :, :, None]
                                    rk = R5[:, k, :, 0:HD].rearrange(
                                        "p t (h f) -> p t h f", f=D)
                                    nc.vector.tensor_tensor(
                                        out=rk, in0=gq,
                                        in1=wq.to_broadcast([128, cfg.t_run, H, D]),
                                        op=mybir.AluOpType.mult)
                            nc.vector.tensor_copy(
                                R5[:, :, :, HD:HD + H],
                                w_all[:].rearrange("p (k t h) -> p k t h",
                                                   k=cfg.n_chunks, h=H))

                            ps = ppool.tile([128, HD + H], F32, tag="ps")
                            for s in range(TPB):
                                nc.tensor.matmul(ps[:],
                                                 lhsT=eq_all[:, s * 128:(s + 1) * 128],
                                                 rhs=R_all[:, s * RC:(s + 1) * RC],
                                                 start=(s == 0), stop=(s == TPB - 1))

                            esum = spool.tile([128, H], F32, tag="esum")
                            nc.vector.tensor_scalar(esum[:], ps[:, HD:HD + H], 1e-30, None,
                                                    mybir.AluOpType.max)
                            rec = spool.tile([128, H], F32, tag="rec")
                            nc.vector.reciprocal(rec[:], esum[:])
                            ob = opool.tile([128, IN_DIM], F32, tag="ob")
                            recb = rec[:][:, :, None]
                            nc.vector.tensor_tensor(
                                out=ob[:].rearrange("p (h f) -> p h f", f=D),
                                in0=ps[:, 0:HD].rearrange("p (h f) -> p h f", f=D),
                                in1=recb.to_broadcast([128, H, D]),
                                op=mybir.AluOpType.mult)
                            nc.vector.tensor_add(ob[:], ob[:], fres[:])
                            nc.sync.dma_start(out[bs, :], ob[:])

    nc.compile()
    return nc


def build_host_data(cfg: Cfg, feat, src, dst, Wc, Wp, attn_src, attn_dst,
                    pos_attn_src, pos_attn_dst, att_comb):
    """Pure index manipulation + data placement (no float math on tensors)."""
    n_pad, bn, chunk, t_run, nch = cfg.n_pad, cfg.bn, cfg.chunk, cfg.t_run, cfg.n_chunks
    nblk = cfg.cores * cfg.nb

    featp = np.zeros((n_pad, IN_DIM), np.float32)
    featp[:cfg.n] = feat
    featT = np.ascontiguousarray(featp.T)

    # sort edges by (dst block, src chunk)
    key = (dst.astype(np.int64) // 128) * nch + (src.astype(np.int64) // chunk)
    order = np.argsort(key, kind='stable')
    s_src = src[order].astype(np.int64)
    s_dst = dst[order].astype(np.int64)
    s_key = key[order]

    counts = np.bincount(s_key, minlength=nblk * nch)
    assert counts.max() <= t_run * 128, f"t_run too small: need {counts.max()}"
    group_start = np.zeros(nblk * nch, np.int64)
    np.cumsum(counts[:-1], out=group_start[1:])
    within = np.arange(len(s_src)) - group_start[s_key]
    slot = s_key * (t_run * 128) + within      # flat slot id

    total_slots = nblk * nch * t_run * 128
    idx_flat = np.zeros(total_slots, np.int16)   # pad slots gather row 0 (zero weight)
    idx_flat[slot] = (s_src - (s_src // chunk) * chunk).astype(np.int16)
    dstl_flat = np.full(total_slots, -1.0, ml_dtypes.bfloat16)
    dstl_flat[slot] = (s_dst - (s_dst // 128) * 128).astype(ml_dtypes.bfloat16)

    # idx grouped by (gather-group, chunk): order (b7, t, p) within a call,
    # then lane layout [128, GT*8]
    GROUP = cfg.group
    NG = cfg.nb // GROUP
    GT8 = GROUP * t_run * 8
    idx_r = (idx_flat.reshape(cfg.cores, NG, GROUP, nch, t_run * 128)
             .transpose(0, 1, 3, 2, 4)           # [cores, NG, nch, GROUP, t*128]
             .reshape(cfg.cores, NG, nch, GROUP * t_run * 128))
    lane = idx_r.reshape(cfg.cores, NG, nch, GT8, 16)
    lane = np.swapaxes(lane, -1, -2)             # [.., 16, GT8]
    lane = np.broadcast_to(lane[:, :, :, None, :, :],
                           (cfg.cores, NG, nch, 8, 16, GT8))
    idx_host = np.ascontiguousarray(
        lane.reshape(cfg.cores, NG, nch, 128, GT8)
            .transpose(0, 1, 3, 2, 4)
            .reshape(cfg.cores, NG, 128, nch * GT8))

    # dstl: slot (k, t, p) -> [.., p, k*t_run + t]
    dstl_r = dstl_flat.reshape(cfg.cores, cfg.nb, nch, t_run, 128)
    dstl_host = np.ascontiguousarray(
        dstl_r.transpose(0, 1, 4, 2, 3)
              .reshape(cfg.cores, cfg.nb, 128, nch * t_run))

    AcatC = np.zeros((HD, 2 * H), np.float32)
    AcatP = np.zeros((H * (D // 4), 2 * H), np.float32)
    for h in range(H):
        AcatC[h * D:(h + 1) * D, h] = attn_src[0, h]
        AcatC[h * D:(h + 1) * D, H + h] = attn_dst[0, h]
        AcatP[h * (D // 4):(h + 1) * (D // 4), h] = pos_attn_src[0, h]
        AcatP[h * (D // 4):(h + 1) * (D // 4), H + h] = pos_attn_dst[0, h]

    a0col = np.tile(att_comb[:, 0], 2).reshape(2 * H, 1).astype(np.float32)
    a1col = np.tile(att_comb[:, 1], 2).reshape(2 * H, 1).astype(np.float32)
    iota_row = np.broadcast_to(np.arange(128, dtype=np.float32), (128, 128)).astype(ml_dtypes.bfloat16)
    ident = np.eye(128, dtype=ml_dtypes.bfloat16)
    identf = np.eye(128, dtype=np.float32)

    in_maps = []
    for c in range(cfg.cores):
        in_maps.append({
            "featT": np.ascontiguousarray(featT[:, c * bn:(c + 1) * bn]),
            "feat_rows": np.ascontiguousarray(featp[c * bn:(c + 1) * bn, :]),
            "idx": idx_host[c],
            "dstl": dstl_host[c],
            "WcT": np.ascontiguousarray(Wc.T),
            "WpT": np.ascontiguousarray(Wp.T),
            "AcatC": AcatC, "AcatP": AcatP,
            "a0col": a0col, "a1col": a1col,
            "iota_row": np.ascontiguousarray(iota_row),
            "ident": ident, "identf": identf,
        })
    return in_maps


def compute_t_run(cfg_nb, cores, n_chunks, src, dst):
    chunk = cores * cfg_nb * 128 // n_chunks
    key = (dst.astype(np.int64) // 128) * n_chunks + (src.astype(np.int64) // chunk)
    counts = np.bincount(key, minlength=cores * cfg_nb * n_chunks)
    return int(-(-counts.max() // 128))


# ---------------------------------------------------------------------------
# Execution (PJRT under axon)

_CACHE = {}


def _get_runner(cfg: Cfg):
    key = (cfg, ABLATE)
    if key in _CACHE:
        return _CACHE[key]
    import jax
    from jax.sharding import Mesh, PartitionSpec, NamedSharding
    from jax.experimental.shard_map import shard_map
    from concourse.bass2jax import _bass_exec_p, install_neuronx_cc_hook, partition_id_tensor

    nc = build_program(cfg)
    install_neuronx_cc_hook()

    partition_name = nc.partition_id_tensor.name if nc.partition_id_tensor else None
    in_names, out_names, out_avals, zero_outs = [], [], [], []
    for alloc in nc.m.functions[0].allocations:
        if not isinstance(alloc, mybir.MemoryLocationSet):
            continue
        name = alloc.memorylocations[0].name
        if alloc.kind == "ExternalInput":
            if name != partition_name:
                in_names.append(name)
        elif alloc.kind == "ExternalOutput":
            out_names.append(name)
            shape = tuple(alloc.tensor_shape)
            dtype = mybir.dt.np(alloc.dtype)
            out_avals.append(jax.core.ShapedArray(shape, dtype))
            zero_outs.append(np.zeros(shape, dtype))
    n_params = len(in_names)
    all_in_names = list(in_names) + list(out_names)
    if partition_name is not None:
        all_in_names.append(partition_name)

    def _body(*args):
        operands = list(args)
        if partition_name is not None:
            operands.append(partition_id_tensor())
        outs = _bass_exec_p.bind(
            *operands, out_avals=tuple(out_avals), in_names=tuple(all_in_names),
            out_names=tuple(out_names), lowering_input_output_aliases=(),
            sim_require_finite=False, sim_require_nnan=False, nc=nc)
        return tuple(outs)

    devices = jax.devices()[:cfg.cores]
    mesh = Mesh(np.asarray(devices), ("core",))
    fn = jax.jit(
        shard_map(_body, mesh=mesh, in_specs=(PartitionSpec("core"),) * (n_params + len(out_names)),
                  out_specs=(PartitionSpec("core"),) * len(out_names), check_rep=False),
        keep_unused=True)
    sharding = NamedSharding(mesh, PartitionSpec("core"))
    state = dict(fn=fn, in_names=in_names, out_names=out_names, out_avals=out_avals,
                 zero_outs=zero_outs, sharding=sharding, cores=cfg.cores)
    _CACHE[key] = state
    return state


def run_on_hw(cfg: Cfg, in_maps):
    import jax
    st = _get_runner(cfg)
    concat_in = [np.concatenate([np.asarray(in_maps[c][nm]) for c in range(st["cores"])], axis=0)
                 for nm in st["in_names"]]
    concat_zero = [np.zeros((st["cores"] * z.shape[0], *z.shape[1:]), z.dtype)
                   for z in st["zero_outs"]]
    dev_args = [jax.device_put(a, st["sharding"]) for a in concat_in + concat_zero]
    outs = st["fn"](*dev_args)
    jax.block_until_ready(outs)
    res = []
    for c in range(st["cores"]):
        d = {}
        for i, nm in enumerate(st["out_names"]):
            full = np.asarray(outs[i])
            d[nm] = full.reshape(st["cores"], *st["out_avals"][i].shape)[c]
        res.append(d)
    return res, dev_args, st


def kernel(feat, src, dst, Wc, Wp, attn_src, attn_dst,
           pos_attn_src, pos_attn_dst, att_comb):
    feat = np.asarray(feat, np.float32)
    src = np.asarray(src, np.int32)
    dst = np.asarray(dst, np.int32)
    Wc = np.asarray(Wc, np.float32)
    Wp = np.asarray(Wp, np.float32)
    attn_src = np.asarray(attn_src, np.float32)
    attn_dst = np.asarray(attn_dst, np.float32)
    pos_attn_src = np.asarray(pos_attn_src, np.float32)
    pos_attn_dst = np.asarray(pos_attn_dst, np.float32)
    att_comb = np.asarray(att_comb, np.float32)

    n = feat.shape[0]
    nb = -(-n // (8 * 128))          # blocks per core (98 for N=100000)
    t_run = compute_t_run(nb, 8, 4, src, dst)
    group = 2 if nb % 2 == 0 else 1
    cfg = Cfg(n=n, nb=nb, t_run=t_run, group=group)

    in_maps = build_host_data(cfg, feat, src, dst, Wc, Wp, attn_src, attn_dst,
                              pos_attn_src, pos_attn_dst, att_comb)
    res, _, _ = run_on_hw(cfg, in_maps)
    h = np.concatenate([res[c]["out"] for c in range(cfg.cores)], axis=0)[:n]
    return h, np.zeros((), np.float32)
